# revision 25
# baseline (speedup 1.0000x reference)
"""Trainium2 Bass kernel for a 2-layer GRU teacher-forced decoder.

Math (per reference):
  toks[t,b]: t=0 -> SOS(=1), t>=1 -> target[b, t]   (T = ML-1 = 63 steps)
  x_t = relu(emb[toks[t]])                          [B, E]
  h0 <- GRUCell(x_t, h0; W_ih0, W_hh0, b_ih0, b_hh0)
  h1 <- GRUCell(h0, h1; W_ih1, W_hh1, b_ih1, b_hh1)
  logits_t = h1 @ W_out.T + b_out                   [B, V]
  out = stack(logits).transpose(1,0,2)              [B, T, V]

Device strategy (8 cores, SPMD, no collectives):
  - The sequential GRU recurrence is replicated on every core; the large
    output projection is sharded column-wise (vocab) 8 ways.
  - Both GRU layers run in ONE software-pipelined loop: iter t does
    layer-0 step t and layer-1 step t-1, so each layer's elementwise tail
    hides under the other layer's PE matmul burst.  h1 history is spilled
    to DRAM so both layers' weights (22 MB bf16) stay SBUF-resident.
  - The logits GEMM runs after the loop, streaming W_out vocab slices
    from DRAM (1 MB double-buffered) and h1 M-blocks from the spill.
  - All matmuls run in bf16; gate elementwise math and state are fp32.

Layouts:
  - Gate tensors live in PSUM as [128, 1024] = [(quarter q, batch b),
    (gate, j)] where hidden = q*256 + j; the 4 hidden-quarters occupy the
    4 PE column groups (tile_position=(0,32q)).  Matmuls are emitted
    k-outer / quarter-inner so the four column-group streams overlap.
  - State h is kept twice: fp32 quarter layout [128, 256] for elementwise,
    and transposed bf16 [128(hidden chunk), 32(batch)] ring tiles used as
    the next matmul's lhsT.
  - Per step the new h is staged to a flat [32, 1024] tile (SBUF->SBUF
    DMA), PE-transposed, and copied to a bf16 ring tile; h1's ring tile is
    additionally spilled to a DRAM history for the logits phase.
"""

import os
import sys
import numpy as np

sys.path.insert(0, "/opt/trn_rl_repo")

import ml_dtypes

V, E, H, B, ML = 32000, 512, 1024, 32, 64
SOS = 1
T = ML - 1          # 63
TB = T * B          # 2016
NCORES = 8
VS = V // NCORES    # 4000 vocab slice per core
Q = 4               # hidden quarters
J = H // Q          # 256
KH = H // 128       # 8 contraction chunks over H
KE = E // 128       # 4 contraction chunks over E
MT = 126            # logits M-tile (2016 = 16 * 126)
NMT = TB // MT      # 16
NS = 500            # logits psum slice width (one 2KB bank)

_BF = ml_dtypes.bfloat16


def _bf16(x):
    return np.asarray(x, np.float32).astype(_BF)


def _prep_wT(w, kchunks):
    """w: [3H, K*128] fp32 -> [128, kchunks, 3H] bf16 with [p, k, col] = w[col, 128k+p]."""
    wt = np.ascontiguousarray(np.asarray(w, np.float32).T)       # [K, 3H]
    wt = wt.reshape(kchunks, 128, wt.shape[1]).transpose(1, 0, 2)  # [128, k, 3H]
    return _bf16(wt)


def _prep_hq(h):
    """h: [B, H] fp32 -> quarter layout [128, 256], [32q+b, j] = h[b, q*256+j]."""
    hq = np.asarray(h, np.float32).reshape(B, Q, J).transpose(1, 0, 2).reshape(Q * B, J)
    return np.ascontiguousarray(hq)


def _prep_hT(h):
    """h: [B, H] -> [128, 8, 32] bf16 with [p, k, b] = h[b, 128k+p]."""
    ht = np.asarray(h, np.float32).T.reshape(KH, 128, B).transpose(1, 0, 2)
    return _bf16(ht)


def _gate_bias_quarter(b_ih, b_hh):
    """Quarter-layout fp32 bias tiles for the DVE adds.

    bq [128, 768]: [32q+b, gate*256+j] = (b_ih+b_hh) for r,z; b_hh for n.
    bc [128, 256]: [32q+b, j] = b_ih n-part.
    """
    bi = np.asarray(b_ih, np.float32)
    bh = np.asarray(b_hh, np.float32)
    comb = np.empty(3 * H, np.float32)
    comb[0:2 * H] = bi[0:2 * H] + bh[0:2 * H]
    comb[2 * H:] = bh[2 * H:]
    bq = np.empty((128, 3 * J), np.float32)
    bc = np.empty((128, J), np.float32)
    for q in range(Q):
        s = q * J
        row = np.concatenate([comb[s:s + J], comb[H + s:H + s + J],
                              comb[2 * H + s:2 * H + s + J]])
        bq[32 * q:32 * (q + 1)] = row[None, :]
        bc[32 * q:32 * (q + 1)] = bi[2 * H + s:2 * H + s + J][None, :]
    return bq, bc


def _build_inputs(encoder_hidden, target_tensor, emb,
                  W_ih0, W_hh0, b_ih0, b_hh0, W_ih1, W_hh1, b_ih1, b_hh1,
                  W_out, b_out):
    """Host-side layout prep. Returns (shared_map, per_core_maps)."""
    tt = np.asarray(target_tensor)
    toks = np.concatenate(
        [np.full((B, 1), SOS, dtype=tt.dtype), tt[:, 1:ML - 1]], axis=1).T  # [T, B]
    X = np.maximum(np.asarray(emb, np.float32)[toks], 0.0)  # [T, B, E]
    # xT [128, KE, T*B]: [p, k, t*32+b] = X[t, b, 128k+p]
    xT = X.reshape(TB, KE, 128).transpose(2, 1, 0)
    xT = np.ascontiguousarray(_bf16(xT))

    # rank-1 bias rows, bf16: biasQ[L][0, q, :] = [rz bias | n-rec bias | n-in]
    def bias_rows(b_ih, b_hh):
        bi = np.asarray(b_ih, np.float32)
        bh = np.asarray(b_hh, np.float32)
        comb = bi[0:2 * H] + bh[0:2 * H]
        bqv = np.empty((1, Q, 4 * J), np.float32)
        for q in range(Q):
            s = q * J
            bqv[0, q, 0:J] = comb[s:s + J]
            bqv[0, q, J:2 * J] = comb[H + s:H + s + J]
            bqv[0, q, 2 * J:3 * J] = bh[2 * H + s:2 * H + s + J]
            bqv[0, q, 3 * J:4 * J] = bi[2 * H + s:2 * H + s + J]
        return _bf16(bqv)

    ident = np.zeros((128, 32), np.float32)
    for g in range(4):
        ident[g * 32:(g + 1) * 32] = np.eye(32, dtype=np.float32)

    shared = {
        "xT": xT,
        "h0q": _prep_hq(encoder_hidden[0]),
        "h1q": _prep_hq(encoder_hidden[1]),
        "h0T": _prep_hT(encoder_hidden[0]),
        "h1T": _prep_hT(encoder_hidden[1]),
        "wih0T": _prep_wT(W_ih0, KE),
        "whh0T": _prep_wT(W_hh0, KH),
        "wih1T": _prep_wT(W_ih1, KH),
        "whh1T": _prep_wT(W_hh1, KH),
        "biasQ0": bias_rows(b_ih0, b_hh0),
        "biasQ1": bias_rows(b_ih1, b_hh1),
        "ones1": _bf16(np.ones((1, 32), np.float32)),
        "ident": ident,
    }
    wout = np.asarray(W_out, np.float32)
    per_core = []
    for c in range(NCORES):
        sl = slice(c * VS, (c + 1) * VS)
        woutT = wout[sl].T.reshape(KH, 128, VS).transpose(1, 0, 2)  # [128, 8, VS]
        per_core.append({
            "woutT": np.ascontiguousarray(_bf16(woutT)),
        })
    return shared, per_core


# ---------------------------------------------------------------------------
# Device program
# ---------------------------------------------------------------------------

def _emit(nc, tc, io, n_steps=T):
    import concourse.bass as bass
    from concourse import mybir
    from concourse.alu_op_type import AluOpType as alu

    f32 = mybir.dt.float32
    bf16 = mybir.dt.bfloat16
    Sig = mybir.ActivationFunctionType.Sigmoid
    Tanh = mybir.ActivationFunctionType.Tanh

    RB = 4  # state-transpose ring depth (per layer)

    ctx_pools = []

    def pool(name, bufs, space="SBUF"):
        p = tc.tile_pool(name=name, bufs=bufs, space=space)
        ctx_pools.append(p)
        return p.__enter__()

    consts = pool("consts", 1)
    arena_p = pool("arena", 1)
    hqp = pool("hq", 2)
    work = pool("work", 2)
    xp = pool("xs", 3)
    stp = pool("stp", 1)
    htp = pool("htp", RB)

    # ---- constants / persistent tensors in SBUF ----
    ident_sb = consts.tile([128, 32], f32)
    nc.sync.dma_start(ident_sb[:], io["ident"][:])
    ones_sb = consts.tile([1, 32], bf16, tag="ones1", name="ones1")
    nc.sync.dma_start(ones_sb[:], io["ones1"][:])
    biasQ = {}
    for L in (0, 1):
        biasQ[L] = consts.tile([1, Q, 4 * J], bf16, tag=f"biasQ{L}",
                               name=f"biasQ{L}")
        nc.sync.dma_start(biasQ[L][:], io[f"biasQ{L}"][:])

    hq_init = {}
    for L in (0, 1):
        hq_init[L] = consts.tile([128, J], f32, tag=f"hq{L}i", name=f"hq{L}i")
        nc.sync.dma_start(hq_init[L][:], io[f"h{L}q"][:])

    # transposed-state ring tiles; slot for S_L(t) kept in python lists
    def new_ht(layer):
        return htp.tile([128, KH, 32], bf16, tag=f"h{layer}T",
                        name=f"h{layer}T")

    hT_cur = {}
    for L in (0, 1):
        t0 = new_ht(L)
        nc.sync.dma_start(t0[:], io[f"h{L}T"][:])
        hT_cur[L] = t0

    # GRU weights, all resident (22 MB bf16)
    a0 = arena_p.tile([128, KE + KH, 3 * H], bf16, tag="a0", name="a0")
    nc.sync.dma_start(a0[:, KE:KE + KH, :], io["whh0T"][:])
    nc.sync.dma_start(a0[:, 0:KE, :], io["wih0T"][:])
    a1 = arena_p.tile([128, 2 * KH, 3 * H], bf16, tag="a1", name="a1")
    nc.sync.dma_start(a1[:, 0:KH, :], io["wih1T"][:])
    nc.sync.dma_start(a1[:, KH:2 * KH, :], io["whh1T"][:])

    def gate_mms_input(psum_pool, gtag, gbufs, Wa, kw, lhsT_of, layer):
        """Bias seeds (rank-1) + input-path (W_ih) matmuls; returns G."""
        G = psum_pool.tile([128, 4 * J], f32, tag=gtag, name=gtag, bufs=gbufs)
        bQ = biasQ[layer]
        for q in range(Q):
            # rank-1 ones x bias row seeds every accumulation region, so the
            # elementwise phase reads fully-biased gates straight from PSUM
            tp = (0, 32 * q)
            nc.tensor.matmul(G[32 * q:32 * q + 32, 0:2 * J],
                             ones_sb[:], bQ[:, q, 0:2 * J],
                             start=True, stop=False, tile_position=tp,
                             skip_group_check=True)
            # one start=True per PSUM bank: start clears has_written for the
            # WHOLE bank, and the n-rec and n-in regions share bank 1
            nc.tensor.matmul(G[32 * q:32 * q + 32, 2 * J:3 * J],
                             ones_sb[:], bQ[:, q, 2 * J:3 * J],
                             start=True, stop=False, tile_position=tp,
                             skip_group_check=True)
            nc.tensor.matmul(G[32 * q:32 * q + 32, 3 * J:4 * J],
                             ones_sb[:], bQ[:, q, 3 * J:4 * J],
                             start=False, stop=False, tile_position=tp,
                             skip_group_check=True)
        for k in range(kw):
            lhsT = lhsT_of(k)
            w3 = Wa[:, k, :].rearrange("p (g j) -> p g j", g=3)
            for q in range(Q):
                tp = (0, 32 * q)
                Gq_rz = G[32 * q:32 * q + 32, 0:2 * J]
                Cq = G[32 * q:32 * q + 32, 3 * J:4 * J]
                nc.tensor.matmul(Gq_rz.rearrange("p (g j) -> p g j", g=2),
                                 lhsT, w3[:, 0:2, q * J:(q + 1) * J],
                                 start=False, stop=False, tile_position=tp,
                                 skip_group_check=True)
                nc.tensor.matmul(Cq, lhsT, w3[:, 2, q * J:(q + 1) * J],
                                 start=False, stop=(k == kw - 1),
                                 tile_position=tp, skip_group_check=True)
        return G

    def gate_mms_rec(G, Wa, wofs, hT_prev):
        """Recurrent-path (W_hh) matmuls; must follow the h state write."""
        for k in range(KH):
            lhsT = hT_prev[:, k, :]
            w3 = Wa[:, wofs + k, :].rearrange("p (g j) -> p g j", g=3)
            for q in range(Q):
                tp = (0, 32 * q)
                Gq_rz = G[32 * q:32 * q + 32, 0:2 * J]
                Gq_n = G[32 * q:32 * q + 32, 2 * J:3 * J]
                nc.tensor.matmul(Gq_rz.rearrange("p (g j) -> p g j", g=2),
                                 lhsT, w3[:, 0:2, q * J:(q + 1) * J],
                                 start=False, stop=(k == KH - 1), tile_position=tp,
                                 skip_group_check=True)
                nc.tensor.matmul(Gq_n, lhsT, w3[:, 2, q * J:(q + 1) * J],
                                 start=False, stop=(k == KH - 1),
                                 tile_position=tp, skip_group_check=True)

    def gate_elem_update(psum_pool, G, hq_prev, layer, spill_col=None):
        """sigmoid/tanh + gated update; returns (hq_new, hT_new ring tile)."""
        # gates in G are fully biased (rank-1 seeds), so read PSUM directly
        Sp = work.tile([128, 2 * J], f32, tag="Sp")
        nc.scalar.activation(Sp[:], G[:, 0:2 * J], Sig)            # r | z
        w2 = work.tile([128, J], f32, tag="w2")
        nc.vector.tensor_tensor(w2[:], Sp[:, 0:J], G[:, 2 * J:3 * J],
                                alu.mult)                          # r*(hn+bhh)
        nc.vector.tensor_tensor(w2[:], G[:, 3 * J:4 * J], w2[:], alu.add)
        nc.scalar.activation(w2[:], w2[:], Tanh)                   # n (in place)
        w3 = work.tile([128, J], f32, tag="w3")
        nc.vector.tensor_tensor(w3[:], Sp[:, J:2 * J], hq_prev[:], alu.mult)
        # reuse the r slot for (1-z); r is dead after the w2 multiply
        nc.vector.tensor_scalar(Sp[:, 0:J], Sp[:, J:2 * J], -1.0, 1.0,
                                alu.mult, alu.add)
        nc.vector.tensor_tensor(w2[:], w2[:], Sp[:, 0:J], alu.mult)  # (1-z)*n
        hq_new = hqp.tile([128, J], f32, tag=f"hq{layer}")
        nc.vector.tensor_tensor(hq_new[:], w2[:], w3[:], alu.add)
        # stage h' quarters into a flat [32, 1024] tile, then PE-transpose
        # from base partition 0 (direct tile_position transposes fault on HW)
        st = stp.tile([32, H], f32, tag="st", name="st")
        for q in range(Q):
            nc.sync.dma_start(st[0:32, q * J:(q + 1) * J],
                              hq_new[32 * q:32 * q + 32, :])
        TP = psum_pool.tile([128, KH * 32], f32, tag="TP", name="TP", bufs=2)
        for k in range(KH):
            nc.tensor.matmul(
                TP[:, 32 * k:32 * k + 32],
                st[0:32, 128 * k:128 * (k + 1)],
                ident_sb[0:32, :],
                is_transpose=True,
                skip_group_check=True,
            )
        hT_new = new_ht(layer)
        nc.scalar.activation(
            hT_new[:],
            TP[:].rearrange("p (k b) -> p k b", k=KH),
            mybir.ActivationFunctionType.Copy,
        )
        if spill_col is not None:
            nc.sync.dma_start(
                io["h1hist"][:, :, spill_col:spill_col + 32], hT_new[:])
        return hq_new, hT_new

    # ================= fused recurrence: L0 step t + L1 step t-1 ============
    hq_cur = {0: hq_init[0], 1: hq_init[1]}

    def load_x(t):
        xt = xp.tile([128, KE, 32], bf16, tag="xt")
        nc.sync.dma_start(xt[:], io["xT"][:, :, t * 32:(t + 1) * 32])
        return xt

    def l1_step(psumG, u):
        """Emit all layer-1 work for step u (input + rec + elem + spill)."""
        G1 = gate_mms_input(psumG, "G1", 1, a1, KH,
                            lambda k: hT_cur[0][:, k, :], 1)
        gate_mms_rec(G1, a1, KH, hT_cur[1])
        hq_cur[1], hT_cur[1] = gate_elem_update(
            psumG, G1, hq_cur[1], 1, spill_col=32 * u)

    with tc.tile_pool(name="psumG", bufs=1, space="PSUM") as psumG:
        xs = [load_x(0)]
        pend = [gate_mms_input(psumG, "G0", 2, a0, KE,
                               lambda k, x=xs[0]: x[:, k, :], 0)]
        for t in range(n_steps):
            # --- layer 0, step t: keep the L0 self-cycle (rec0 -> elem0 ->
            # transpose -> rec0) at the head of the PE queue; layer-1 MMs
            # come after and fill the PE idle time under layer-0's tail ---
            G0 = pend.pop(0)
            h0T_prev = hT_cur[0]
            gate_mms_rec(G0, a0, KE, h0T_prev)
            if t + 1 < n_steps:
                xs.append(load_x(t + 1))
                pend.append(gate_mms_input(psumG, "G0", 2, a0, KE,
                                           lambda k, x=xs[t + 1]: x[:, k, :], 0))
            hq_cur[0], hT_cur[0] = gate_elem_update(psumG, G0, hq_cur[0], 0)
            # --- layer 1, step t-1 (inputs S0(t)=h0T_prev, S1(t-1) ready) ---
            if t >= 1:
                save = hT_cur[0]
                hT_cur[0] = h0T_prev
                l1_step(psumG, t - 1)
                hT_cur[0] = save
        # final layer-1 step uses the last h0 state (hT_cur[0] = S0(T))
        l1_step(psumG, n_steps - 1)

    # close recurrence pools to free SBUF for the logits phase
    for p in reversed(ctx_pools):
        p.__exit__(None, None, None)
    ctx_pools.clear()

    # ================= logits GEMM (vocab-sharded, W_out streamed) ==========
    n_rows = n_steps * B
    n_mt = (n_rows + MT - 1) // MT
    with tc.tile_pool(name="wop", bufs=2) as wop, \
         tc.tile_pool(name="hbp", bufs=3) as hbp, \
         tc.tile_pool(name="psumL", bufs=4, space="PSUM") as psumL, \
         tc.tile_pool(name="outp", bufs=4) as outp:
        for s in range(VS // NS):
            ws = wop.tile([128, KH, NS], bf16, tag="ws")
            nc.sync.dma_start(ws[:], io["woutT"][:, :, s * NS:(s + 1) * NS])
            for m in range(n_mt):
                rows = min(MT, n_rows - m * MT)
                hb = hbp.tile([128, KH, MT], bf16, tag="hb")
                nc.sync.dma_start(
                    hb[:, :, 0:rows],
                    io["h1hist"][:, :, m * MT:m * MT + rows])
                L = psumL.tile([128, NS], f32, tag="L", name="L")
                for k in range(KH):
                    nc.tensor.matmul(
                        L[0:rows, :],
                        hb[:, k, 0:rows],
                        ws[:, k, :],
                        start=(k == 0), stop=(k == KH - 1))
                ob = outp.tile([128, NS], f32, tag="ob", name="ob")
                nc.vector.tensor_copy(ob[0:rows, :], L[0:rows, :])
                nc.sync.dma_start(
                    io["logits"][m * MT:m * MT + rows, s * NS:(s + 1) * NS],
                    ob[0:rows, :])


def _build_program(n_steps=T):
    import concourse.bacc as bacc
    import concourse.tile as tile
    from concourse import mybir

    f32 = mybir.dt.float32
    bf16 = mybir.dt.bfloat16

    nc = bacc.Bacc("TRN2", target_bir_lowering=False, debug=False,
                   num_devices=NCORES)

    def din(name, shape, dt):
        return nc.dram_tensor(name, list(shape), dt, kind="ExternalInput").ap()

    io = {
        "xT": din("xT", (128, KE, TB), bf16),
        "h0q": din("h0q", (128, J), f32),
        "h1q": din("h1q", (128, J), f32),
        "h0T": din("h0T", (128, KH, 32), bf16),
        "h1T": din("h1T", (128, KH, 32), bf16),
        "wih0T": din("wih0T", (128, KE, 3 * H), bf16),
        "whh0T": din("whh0T", (128, KH, 3 * H), bf16),
        "wih1T": din("wih1T", (128, KH, 3 * H), bf16),
        "whh1T": din("whh1T", (128, KH, 3 * H), bf16),
        "biasQ0": din("biasQ0", (1, Q, 4 * J), bf16),
        "biasQ1": din("biasQ1", (1, Q, 4 * J), bf16),
        "ones1": din("ones1", (1, 32), bf16),
        "ident": din("ident", (128, 32), f32),
        "woutT": din("woutT", (128, KH, VS), bf16),
        "h1hist": nc.dram_tensor("h1hist", [128, KH, TB], bf16,
                                 kind="Internal").ap(),
        "logits": nc.dram_tensor("logits", [TB, VS], f32,
                                 kind="ExternalOutput").ap(),
    }

    with tile.TileContext(nc) as tc:
        _emit(nc, tc, io, n_steps=n_steps)

    nc.compile()
    return nc


_CACHED = {}


def _get_program(n_steps=T):
    if n_steps not in _CACHED:
        _CACHED[n_steps] = _build_program(n_steps)
    return _CACHED[n_steps]


def kernel(encoder_outputs, encoder_hidden, target_tensor, emb,
           W_ih0, W_hh0, b_ih0, b_hh0, W_ih1, W_hh1, b_ih1, b_hh1,
           W_out, b_out, _trace=False):
    from concourse import bass_utils

    shared, per_core = _build_inputs(
        encoder_hidden, target_tensor, emb,
        W_ih0, W_hh0, b_ih0, b_hh0, W_ih1, W_hh1, b_ih1, b_hh1, W_out, b_out)

    nc = _get_program()
    in_maps = []
    for c in range(NCORES):
        m = dict(shared)
        m.update(per_core[c])
        in_maps.append(m)

    res = None
    for attempt in range(3):
        try:
            res = bass_utils.run_bass_kernel_spmd(
                nc, in_maps, core_ids=list(range(NCORES)), trace=_trace)
            break
        except Exception:
            if attempt == 2:
                raise
            import time
            time.sleep(20)

    parts = [res.results[c]["logits"].reshape(T, B, VS) for c in range(NCORES)]
    full = np.concatenate(parts, axis=2)          # [T, B, V]
    full += np.asarray(b_out, np.float32)[None, None, :]
    out = np.ascontiguousarray(full.transpose(1, 0, 2)).astype(np.float32)
    if _trace:
        kernel.last_results = res
    return out


kernel.last_results = None


# revision 32
# speedup vs baseline: 1.2178x; 1.2178x over previous
"""Trainium2 Bass kernel for a 2-layer GRU teacher-forced decoder.

Math (per reference):
  toks[t,b]: t=0 -> SOS(=1), t>=1 -> target[b, t]   (T = ML-1 = 63 steps)
  x_t = relu(emb[toks[t]])                          [B, E]
  h0 <- GRUCell(x_t, h0; W_ih0, W_hh0, b_ih0, b_hh0)
  h1 <- GRUCell(h0, h1; W_ih1, W_hh1, b_ih1, b_hh1)
  logits_t = h1 @ W_out.T + b_out                   [B, V]
  out = stack(logits).transpose(1,0,2)              [B, T, V]

Device strategy (8 cores, SPMD, no collectives):
  - The sequential GRU recurrence is replicated on every core; the large
    output projection is sharded column-wise (vocab) 8 ways.
  - Both GRU layers run in ONE software-pipelined loop: iter t does
    layer-0 step t and layer-1 step t-1, so each layer's elementwise tail
    hides under the other layer's PE matmul burst.  h1 history is spilled
    to DRAM so both layers' weights (22 MB bf16) stay SBUF-resident.
  - The logits GEMM runs after the loop, streaming W_out vocab slices
    from DRAM (1 MB double-buffered) and h1 M-blocks from the spill.
  - All matmuls run in bf16; gate elementwise math and state are fp32.

Layouts:
  - Gate tensors live in PSUM as [128, 1024] = [(quarter q, batch b),
    (gate, j)] where hidden = q*256 + j; the 4 hidden-quarters occupy the
    4 PE column groups (tile_position=(0,32q)).  Matmuls are emitted
    k-outer / quarter-inner so the four column-group streams overlap.
  - State h is kept twice: fp32 quarter layout [128, 256] for elementwise,
    and transposed bf16 [128(hidden chunk), 32(batch)] ring tiles used as
    the next matmul's lhsT.
  - Per step the new h is staged to a flat [32, 1024] tile (SBUF->SBUF
    DMA), PE-transposed, and copied to a bf16 ring tile; h1's ring tile is
    additionally spilled to a DRAM history for the logits phase.
"""

import os
import sys
import numpy as np

sys.path.insert(0, "/opt/trn_rl_repo")

import ml_dtypes

V, E, H, B, ML = 32000, 512, 1024, 32, 64
SOS = 1
T = ML - 1          # 63
TB = T * B          # 2016
NCORES = 8
VS = V // NCORES    # 4000 vocab slice per core
Q = 4               # hidden quarters
J = H // Q          # 256
KH = H // 128       # 8 contraction chunks over H
KE = E // 128       # 4 contraction chunks over E
MT = 126            # logits M-tile (2016 = 16 * 126)
NMT = TB // MT      # 16
NS = 500            # logits psum slice width (one 2KB bank)

_BF = ml_dtypes.bfloat16


def _bf16(x):
    return np.asarray(x, np.float32).astype(_BF)


def _prep_wT(w, kchunks):
    """w: [3H, K*128] fp32 -> [128, kchunks, 3H] bf16 with [p, k, col] = w[col, 128k+p]."""
    wt = np.ascontiguousarray(np.asarray(w, np.float32).T)       # [K, 3H]
    wt = wt.reshape(kchunks, 128, wt.shape[1]).transpose(1, 0, 2)  # [128, k, 3H]
    return _bf16(wt)


def _prep_hq(h):
    """h: [B, H] fp32 -> quarter layout [128, 256], [32q+b, j] = h[b, q*256+j]."""
    hq = np.asarray(h, np.float32).reshape(B, Q, J).transpose(1, 0, 2).reshape(Q * B, J)
    return np.ascontiguousarray(hq)


def _prep_hT(h):
    """h: [B, H] -> [128, 8, 32] bf16 with [p, k, b] = h[b, 128k+p]."""
    ht = np.asarray(h, np.float32).T.reshape(KH, 128, B).transpose(1, 0, 2)
    return _bf16(ht)


def _gate_bias_quarter(b_ih, b_hh):
    """Quarter-layout fp32 bias tiles for the DVE adds.

    bq [128, 768]: [32q+b, gate*256+j] = (b_ih+b_hh) for r,z; b_hh for n.
    bc [128, 256]: [32q+b, j] = b_ih n-part.
    """
    bi = np.asarray(b_ih, np.float32)
    bh = np.asarray(b_hh, np.float32)
    comb = np.empty(3 * H, np.float32)
    comb[0:2 * H] = bi[0:2 * H] + bh[0:2 * H]
    comb[2 * H:] = bh[2 * H:]
    bq = np.empty((128, 3 * J), np.float32)
    bc = np.empty((128, J), np.float32)
    for q in range(Q):
        s = q * J
        row = np.concatenate([comb[s:s + J], comb[H + s:H + s + J],
                              comb[2 * H + s:2 * H + s + J]])
        bq[32 * q:32 * (q + 1)] = row[None, :]
        bc[32 * q:32 * (q + 1)] = bi[2 * H + s:2 * H + s + J][None, :]
    return bq, bc


def _build_inputs(encoder_hidden, target_tensor, emb,
                  W_ih0, W_hh0, b_ih0, b_hh0, W_ih1, W_hh1, b_ih1, b_hh1,
                  W_out, b_out):
    """Host-side layout prep. Returns (shared_map, per_core_maps)."""
    tt = np.asarray(target_tensor)
    toks = np.concatenate(
        [np.full((B, 1), SOS, dtype=tt.dtype), tt[:, 1:ML - 1]], axis=1).T  # [T, B]
    X = np.maximum(np.asarray(emb, np.float32)[toks], 0.0)  # [T, B, E]
    # xT [128, KE, T*B]: [p, k, t*32+b] = X[t, b, 128k+p]
    xT = X.reshape(TB, KE, 128).transpose(2, 1, 0)
    xT = np.ascontiguousarray(_bf16(xT))

    # rank-1 bias rows, bf16: biasQ[L][0, q, :] = [rz bias | n-rec bias | n-in]
    def bias_rows(b_ih, b_hh):
        bi = np.asarray(b_ih, np.float32)
        bh = np.asarray(b_hh, np.float32)
        comb = bi[0:2 * H] + bh[0:2 * H]
        bqv = np.empty((1, Q, 4 * J), np.float32)
        for q in range(Q):
            s = q * J
            bqv[0, q, 0:J] = comb[s:s + J]
            bqv[0, q, J:2 * J] = comb[H + s:H + s + J]
            bqv[0, q, 2 * J:3 * J] = bh[2 * H + s:2 * H + s + J]
            bqv[0, q, 3 * J:4 * J] = bi[2 * H + s:2 * H + s + J]
        return _bf16(bqv)

    ident = np.zeros((128, 32), np.float32)
    for g in range(4):
        ident[g * 32:(g + 1) * 32] = np.eye(32, dtype=np.float32)

    shared = {
        "xT": xT,
        "h0q": _prep_hq(encoder_hidden[0]),
        "h1q": _prep_hq(encoder_hidden[1]),
        "h0T": _prep_hT(encoder_hidden[0]),
        "h1T": _prep_hT(encoder_hidden[1]),
        "wih0T": _prep_wT(W_ih0, KE),
        "whh0T": _prep_wT(W_hh0, KH),
        "wih1T": _prep_wT(W_ih1, KH),
        "whh1T": _prep_wT(W_hh1, KH),
        "biasQ0": bias_rows(b_ih0, b_hh0),
        "biasQ1": bias_rows(b_ih1, b_hh1),
        "ones1": _bf16(np.ones((1, 32), np.float32)),
        "identb": _bf16(ident),
    }
    wout = np.asarray(W_out, np.float32)
    per_core = []
    for c in range(NCORES):
        sl = slice(c * VS, (c + 1) * VS)
        woutT = wout[sl].T.reshape(KH, 128, VS).transpose(1, 0, 2)  # [128, 8, VS]
        per_core.append({
            "woutT": np.ascontiguousarray(_bf16(woutT)),
        })
    return shared, per_core


# ---------------------------------------------------------------------------
# Device program
# ---------------------------------------------------------------------------

def _emit(nc, tc, io, n_steps=T):
    import concourse.bass as bass
    from concourse import mybir
    from concourse.alu_op_type import AluOpType as alu

    f32 = mybir.dt.float32
    bf16 = mybir.dt.bfloat16
    Sig = mybir.ActivationFunctionType.Sigmoid
    Tanh = mybir.ActivationFunctionType.Tanh

    RB = 3  # state-transpose ring depth (per layer)

    ctx_pools = []

    def pool(name, bufs, space="SBUF"):
        p = tc.tile_pool(name=name, bufs=bufs, space=space)
        ctx_pools.append(p)
        return p.__enter__()

    consts = pool("consts", 1)
    arena_p = pool("arena", 1)
    hqp = pool("hq", 2)
    work = pool("work", 2)
    xp = pool("xs", 3)
    stp = pool("stp", 1)
    htp = pool("htp", RB)

    # ---- constants / persistent tensors in SBUF ----
    identb_sb = consts.tile([128, 32], bf16, tag="identb", name="identb")
    nc.sync.dma_start(identb_sb[:], io["identb"][:])
    ones_sb = consts.tile([1, 32], bf16, tag="ones1", name="ones1")
    nc.sync.dma_start(ones_sb[:], io["ones1"][:])
    biasQ = {}
    for L in (0, 1):
        biasQ[L] = consts.tile([1, Q, 4 * J], bf16, tag=f"biasQ{L}",
                               name=f"biasQ{L}")
        nc.sync.dma_start(biasQ[L][:], io[f"biasQ{L}"][:])

    hq_init = {}
    for L in (0, 1):
        hq_init[L] = consts.tile([128, J], f32, tag=f"hq{L}i", name=f"hq{L}i")
        nc.sync.dma_start(hq_init[L][:], io[f"h{L}q"][:])

    # transposed-state ring tiles; slot for S_L(t) kept in python lists
    def new_ht(layer):
        return htp.tile([128, KH, 32], bf16, tag=f"h{layer}T",
                        name=f"h{layer}T")

    hT_cur = {}
    for L in (0, 1):
        t0 = new_ht(L)
        nc.sync.dma_start(t0[:], io[f"h{L}T"][:])
        hT_cur[L] = t0

    # GRU weights, all resident (22 MB bf16)
    a0 = arena_p.tile([128, KE + KH, 3 * H], bf16, tag="a0", name="a0")
    nc.sync.dma_start(a0[:, KE:KE + KH, :], io["whh0T"][:])
    nc.sync.dma_start(a0[:, 0:KE, :], io["wih0T"][:])
    a1 = arena_p.tile([128, 2 * KH, 3 * H], bf16, tag="a1", name="a1")
    nc.sync.dma_start(a1[:, 0:KH, :], io["wih1T"][:])
    nc.sync.dma_start(a1[:, KH:2 * KH, :], io["whh1T"][:])

    def gate_mms_input(psum_pool, gtag, gbufs, Wa, kw, lhsT_of, layer):
        """Bias seeds (rank-1) + input-path (W_ih) matmuls; returns G."""
        G = psum_pool.tile([128, 4 * J], f32, tag=gtag, name=gtag, bufs=gbufs)
        bQ = biasQ[layer]
        for q in range(Q):
            # rank-1 ones x bias row seeds every accumulation region, so the
            # elementwise phase reads fully-biased gates straight from PSUM
            tp = (0, 32 * q)
            nc.tensor.matmul(G[32 * q:32 * q + 32, 0:2 * J],
                             ones_sb[:], bQ[:, q, 0:2 * J],
                             start=True, stop=False, tile_position=tp,
                             skip_group_check=True)
            # one start=True per PSUM bank: start clears has_written for the
            # WHOLE bank, and the n-rec and n-in regions share bank 1
            nc.tensor.matmul(G[32 * q:32 * q + 32, 2 * J:3 * J],
                             ones_sb[:], bQ[:, q, 2 * J:3 * J],
                             start=True, stop=False, tile_position=tp,
                             skip_group_check=True)
            nc.tensor.matmul(G[32 * q:32 * q + 32, 3 * J:4 * J],
                             ones_sb[:], bQ[:, q, 3 * J:4 * J],
                             start=False, stop=False, tile_position=tp,
                             skip_group_check=True)
        for k in range(kw):
            lhsT = lhsT_of(k)
            w3 = Wa[:, k, :].rearrange("p (g j) -> p g j", g=3)
            for q in range(Q):
                tp = (0, 32 * q)
                Gq_rz = G[32 * q:32 * q + 32, 0:2 * J]
                Cq = G[32 * q:32 * q + 32, 3 * J:4 * J]
                nc.tensor.matmul(Gq_rz.rearrange("p (g j) -> p g j", g=2),
                                 lhsT, w3[:, 0:2, q * J:(q + 1) * J],
                                 start=False, stop=False, tile_position=tp,
                                 skip_group_check=True)
                nc.tensor.matmul(Cq, lhsT, w3[:, 2, q * J:(q + 1) * J],
                                 start=False, stop=(k == kw - 1),
                                 tile_position=tp, skip_group_check=True)
        return G

    def gate_mms_rec(G, Wa, wofs, hT_prev):
        """Recurrent-path (W_hh) matmuls; must follow the h state write."""
        for k in range(KH):
            lhsT = hT_prev[:, k, :]
            w3 = Wa[:, wofs + k, :].rearrange("p (g j) -> p g j", g=3)
            for q in range(Q):
                tp = (0, 32 * q)
                Gq_rz = G[32 * q:32 * q + 32, 0:2 * J]
                Gq_n = G[32 * q:32 * q + 32, 2 * J:3 * J]
                nc.tensor.matmul(Gq_rz.rearrange("p (g j) -> p g j", g=2),
                                 lhsT, w3[:, 0:2, q * J:(q + 1) * J],
                                 start=False, stop=(k == KH - 1), tile_position=tp,
                                 skip_group_check=True)
                nc.tensor.matmul(Gq_n, lhsT, w3[:, 2, q * J:(q + 1) * J],
                                 start=False, stop=(k == KH - 1),
                                 tile_position=tp, skip_group_check=True)

    def gate_elem(psum_pool, G, hq_prev, layer):
        """sigmoid/tanh + gated update; returns (hq_new, staged bf16 tile)."""
        # gates in G are fully biased (rank-1 seeds), so read PSUM directly
        Sp = work.tile([128, 2 * J], f32, tag="Sp")
        nc.scalar.activation(Sp[:], G[:, 0:2 * J], Sig)            # r | z
        w2 = work.tile([128, J], f32, tag="w2")
        nc.vector.tensor_tensor(w2[:], Sp[:, 0:J], G[:, 2 * J:3 * J],
                                alu.mult)                          # r*(hn+bhh)
        nc.vector.tensor_tensor(w2[:], G[:, 3 * J:4 * J], w2[:], alu.add)
        nc.scalar.activation(w2[:], w2[:], Tanh)                   # n (in place)
        w3 = work.tile([128, J], f32, tag="w3")
        nc.vector.tensor_tensor(w3[:], Sp[:, J:2 * J], hq_prev[:], alu.mult)
        # reuse the r slot for (1-z); r is dead after the w2 multiply
        nc.vector.tensor_scalar(Sp[:, 0:J], Sp[:, J:2 * J], -1.0, 1.0,
                                alu.mult, alu.add)
        nc.vector.tensor_tensor(w2[:], w2[:], Sp[:, 0:J], alu.mult)  # (1-z)*n
        hq_new = hqp.tile([128, J], f32, tag=f"hq{layer}")
        nc.vector.tensor_tensor(hq_new[:], w2[:], w3[:], alu.add)
        # bf16 copy (the transposed state is bf16 anyway), then flatten the
        # quarters to a [32, 1024] tile via DMAs on otherwise-idle queues
        hqb = work.tile([128, J], bf16, tag=f"hqb{layer}", bufs=1)
        nc.scalar.activation(hqb[:], hq_new[:],
                             mybir.ActivationFunctionType.Copy)
        st = stp.tile([32, H], bf16, tag=f"st{layer}", name="st")
        for q in range(Q):
            eng = nc.gpsimd if q % 2 == 0 else nc.scalar
            eng.dma_start(st[0:32, q * J:(q + 1) * J],
                          hqb[32 * q:32 * q + 32, :])
        return hq_new, st

    def gate_transpose(psum_pool, st, layer, spill_col=None):
        """PE-transpose the staged state; returns the bf16 hT ring tile."""
        TP = psum_pool.tile([128, KH * 32], bf16, tag="TP", name="TP", bufs=2)
        for k in range(KH):
            nc.tensor.matmul(
                TP[:, 32 * k:32 * k + 32],
                st[0:32, 128 * k:128 * (k + 1)],
                identb_sb[0:32, :],
                is_transpose=True,
                skip_group_check=True,
            )
        hT_new = new_ht(layer)
        nc.scalar.activation(
            hT_new[:],
            TP[:].rearrange("p (k b) -> p k b", k=KH),
            mybir.ActivationFunctionType.Copy,
        )
        if spill_col is not None:
            nc.sync.dma_start(
                io["h1hist"][:, :, spill_col:spill_col + 32], hT_new[:])
        return hT_new

    # ================= fused recurrence: L0 step t + L1 step t-1 ============
    # Emission per iteration (PE FIFO): rec0(t) | input0(t+1) | gi1(t-1) |
    # transp0(t) | rec1(t-1) | transp1(t-1).  Each PE item's dependencies
    # resolve either PE-internally or from the previous iteration, so the
    # queue never parks on this iteration's elementwise tails.
    hq_cur = {0: hq_init[0], 1: hq_init[1]}

    def load_x(t):
        xt = xp.tile([128, KE, 32], bf16, tag="xt")
        nc.sync.dma_start(xt[:], io["xT"][:, :, t * 32:(t + 1) * 32])
        return xt

    with tc.tile_pool(name="psumG", bufs=1, space="PSUM") as psumG:
        xs = [load_x(0)]
        pend = [gate_mms_input(psumG, "G0", 2, a0, KE,
                               lambda k, x=xs[0]: x[:, k, :], 0)]
        for t in range(n_steps):
            # layer 0, step t
            G0 = pend.pop(0)
            gate_mms_rec(G0, a0, KE, hT_cur[0])
            if t + 1 < n_steps:
                xs.append(load_x(t + 1))
                pend.append(gate_mms_input(psumG, "G0", 2, a0, KE,
                                           lambda k, x=xs[t + 1]: x[:, k, :], 0))
            hq_cur[0], st0 = gate_elem(psumG, G0, hq_cur[0], 0)
            # layer 1, step t-1: input MMs read S0(t) (pre-update hT_cur[0])
            G1 = None
            if t >= 1:
                h0T_l1 = hT_cur[0]
                G1 = gate_mms_input(psumG, "G1", 1, a1, KH,
                                    lambda k: h0T_l1[:, k, :], 1)
            hT_cur[0] = gate_transpose(psumG, st0, 0)
            if t >= 1:
                gate_mms_rec(G1, a1, KH, hT_cur[1])
                hq_cur[1], st1 = gate_elem(psumG, G1, hq_cur[1], 1)
                hT_cur[1] = gate_transpose(psumG, st1, 1, spill_col=32 * (t - 1))
        # final layer-1 step uses the last h0 state (hT_cur[0] = S0(T))
        G1 = gate_mms_input(psumG, "G1", 1, a1, KH,
                            lambda k: hT_cur[0][:, k, :], 1)
        gate_mms_rec(G1, a1, KH, hT_cur[1])
        hq_cur[1], st1 = gate_elem(psumG, G1, hq_cur[1], 1)
        hT_cur[1] = gate_transpose(psumG, st1, 1, spill_col=32 * (n_steps - 1))

    # close recurrence pools to free SBUF for the logits phase
    for p in reversed(ctx_pools):
        p.__exit__(None, None, None)
    ctx_pools.clear()

    # ================= logits GEMM (vocab-sharded, W_out streamed) ==========
    n_rows = n_steps * B
    n_mt = (n_rows + MT - 1) // MT
    with tc.tile_pool(name="wop", bufs=2) as wop, \
         tc.tile_pool(name="hbp", bufs=3) as hbp, \
         tc.tile_pool(name="psumL", bufs=4, space="PSUM") as psumL, \
         tc.tile_pool(name="outp", bufs=4) as outp:
        for s in range(VS // NS):
            ws = wop.tile([128, KH, NS], bf16, tag="ws")
            nc.sync.dma_start(ws[:], io["woutT"][:, :, s * NS:(s + 1) * NS])
            for m in range(n_mt):
                rows = min(MT, n_rows - m * MT)
                hb = hbp.tile([128, KH, MT], bf16, tag="hb")
                nc.sync.dma_start(
                    hb[:, :, 0:rows],
                    io["h1hist"][:, :, m * MT:m * MT + rows])
                L = psumL.tile([128, NS], f32, tag="L", name="L")
                for k in range(KH):
                    nc.tensor.matmul(
                        L[0:rows, :],
                        hb[:, k, 0:rows],
                        ws[:, k, :],
                        start=(k == 0), stop=(k == KH - 1))
                ob = outp.tile([128, NS], f32, tag="ob", name="ob")
                nc.vector.tensor_copy(ob[0:rows, :], L[0:rows, :])
                nc.sync.dma_start(
                    io["logits"][m * MT:m * MT + rows, s * NS:(s + 1) * NS],
                    ob[0:rows, :])


def _build_program(n_steps=T):
    import concourse.bacc as bacc
    import concourse.tile as tile
    from concourse import mybir

    f32 = mybir.dt.float32
    bf16 = mybir.dt.bfloat16

    nc = bacc.Bacc("TRN2", target_bir_lowering=False, debug=False,
                   num_devices=NCORES)

    def din(name, shape, dt):
        return nc.dram_tensor(name, list(shape), dt, kind="ExternalInput").ap()

    io = {
        "xT": din("xT", (128, KE, TB), bf16),
        "h0q": din("h0q", (128, J), f32),
        "h1q": din("h1q", (128, J), f32),
        "h0T": din("h0T", (128, KH, 32), bf16),
        "h1T": din("h1T", (128, KH, 32), bf16),
        "wih0T": din("wih0T", (128, KE, 3 * H), bf16),
        "whh0T": din("whh0T", (128, KH, 3 * H), bf16),
        "wih1T": din("wih1T", (128, KH, 3 * H), bf16),
        "whh1T": din("whh1T", (128, KH, 3 * H), bf16),
        "biasQ0": din("biasQ0", (1, Q, 4 * J), bf16),
        "biasQ1": din("biasQ1", (1, Q, 4 * J), bf16),
        "ones1": din("ones1", (1, 32), bf16),
        "identb": din("identb", (128, 32), bf16),
        "woutT": din("woutT", (128, KH, VS), bf16),
        "h1hist": nc.dram_tensor("h1hist", [128, KH, TB], bf16,
                                 kind="Internal").ap(),
        "logits": nc.dram_tensor("logits", [TB, VS], f32,
                                 kind="ExternalOutput").ap(),
    }

    with tile.TileContext(nc) as tc:
        _emit(nc, tc, io, n_steps=n_steps)

    nc.compile()
    return nc


_CACHED = {}


def _get_program(n_steps=T):
    if n_steps not in _CACHED:
        _CACHED[n_steps] = _build_program(n_steps)
    return _CACHED[n_steps]


def kernel(encoder_outputs, encoder_hidden, target_tensor, emb,
           W_ih0, W_hh0, b_ih0, b_hh0, W_ih1, W_hh1, b_ih1, b_hh1,
           W_out, b_out, _trace=False):
    from concourse import bass_utils

    shared, per_core = _build_inputs(
        encoder_hidden, target_tensor, emb,
        W_ih0, W_hh0, b_ih0, b_hh0, W_ih1, W_hh1, b_ih1, b_hh1, W_out, b_out)

    nc = _get_program()
    in_maps = []
    for c in range(NCORES):
        m = dict(shared)
        m.update(per_core[c])
        in_maps.append(m)

    res = None
    for attempt in range(3):
        try:
            res = bass_utils.run_bass_kernel_spmd(
                nc, in_maps, core_ids=list(range(NCORES)), trace=_trace)
            break
        except Exception:
            if attempt == 2:
                raise
            import time
            time.sleep(20)

    parts = [res.results[c]["logits"].reshape(T, B, VS) for c in range(NCORES)]
    full = np.concatenate(parts, axis=2)          # [T, B, V]
    full += np.asarray(b_out, np.float32)[None, None, :]
    out = np.ascontiguousarray(full.transpose(1, 0, 2)).astype(np.float32)
    if _trace:
        kernel.last_results = res
    return out


kernel.last_results = None
